# revision 11
# baseline (speedup 1.0000x reference)
"""MoE expert FFN (forward_all + top-2 routing combine) on 8 TRN2 NeuronCores.

Strategy: the routing tensor has exactly TOP_K=2 nonzeros per token, so only
routed (token, expert) pairs contribute. We dispatch on the host and run the
expert GEMMs expert-parallel on 8 cores. The device program is a sequence of
K "phases"; each phase is a (weights, tokens) GEMM pair of a fixed compiled
capacity, and the host assigns any expert's weights + routed tokens to any
(core, phase) slot. Because the program is compiled per observed routing
counts (cached by capacity signature), experts can be SPLIT across slots and
capacities chosen so padding is ~1% instead of padding every expert to a
global cap. A small solver cuts each expert into two pieces and groups the
8*K pieces by size rank, minimizing the sum of group maxima.

Each phase computes y^T = gate * (w2^T @ gelu(w1^T @ x^T + b1)) with tokens
on the matmul free dim, f16 matmuls (f32 PSUM accumulate, bias+gelu in f32,
gate-mul on DVE), k-interleaved over 8 PSUM banks. Stage 1 runs over ALL of
a phase's chunks before stage 2 starts, so the w1 stream has the whole
stage-1 span to land and stage-2's w2 streams during stage 1. All matmul
operands are 2-D full-tile slices (3-D mid-tensor slices measurably slow the
PE by ~20%). Phase 0's w1 half-columns + per-ko x interleave on the Sync DGE
in consumption order; later phases prefetch in the background (x/gates/b1 on
the Scalar DGE, weights on Sync) dep-chained behind the previous phase's
weight stream. A short warmup matmul block ramps the PE p-state during the
initial DMA wait. PSUM tiles are allocated full-bank so matmul outputs never
straddle banks. y is written f16.
"""

import math
from contextlib import ExitStack

import numpy as np

import concourse.mybir as mybir
import concourse.tile as tile
from concourse import bacc
from concourse.bass_utils import run_bass_kernel_spmd

N, DIM, E, EXPERT_DIM = 8192, 1024, 16, 2048
N_CORES = 8
P = 128

KO1 = DIM // P  # 8 contraction tiles, stage 1
MO1 = EXPERT_DIM // P  # 16 output tiles, stage 1
KO2 = EXPERT_DIM // P  # 16 contraction tiles, stage 2
MO2 = DIM // P  # 8 output tiles, stage 2

GRP = 8  # stage-1 psum group = all 8 PSUM banks
MIN_CHUNK = 240  # below this, ldweights (~97ns) outruns the matmul free dim

TRACE = False  # set by test.py to capture an NTFF profile
LAST_EXEC_NS = None
LAST_TRACE_PATH = None
ACT_FUNC = None  # default Gelu; sim_check overrides (CoreSim lacks Gelu)
WARMUP_MM = 8  # matmuls ramping the PE p-state during the startup DMA wait

_NC_CACHE = {}


def _chunks_for(cap, last_phase, first_phase=False):
    """Split a phase capacity into matmul chunks <=512 (one PSUM bank of f32),
    each >=MIN_CHUNK when possible. Phase 0 gets a maximal first chunk (the
    bigger the chunk, the slower stage 1 consumes each w1 k-slice, covering
    the DMA ramp). The very last chunk of the last phase is kept small-ish so
    the post-matmul tail (gate-mul + y DMA) is short."""
    if cap <= 512:
        return [cap]
    if first_phase and cap >= 512 + MIN_CHUNK:
        rest = cap - 512
        n = max(1, -(-rest // 512))
        while n > 1 and rest // n < MIN_CHUNK:
            n -= 1
        ch = [512] + [rest // n + (1 if i < rest % n else 0) for i in range(n)]
        if min(ch) >= MIN_CHUNK:
            return ch
    n = -(-cap // 512)  # ceil
    while True:
        base = cap // n
        if base >= MIN_CHUNK or n == 2:
            break
        n -= 1
    ch = [base + (1 if i < cap % n else 0) for i in range(n)]
    ch.sort(reverse=True)
    if last_phase and len(ch) > 1:
        tail = min(ch[-1], 256)
        rest = cap - tail
        m = len(ch) - 1
        ch = [rest // m + (1 if i < rest % m else 0) for i in range(m)]
        ch.sort(reverse=True)
        ch.append(tail)
    return ch


def _build_nc(caps):
    """caps: tuple of per-phase token capacities (descending-ish)."""
    f32 = mybir.dt.float32
    f16 = mybir.dt.float16
    K = len(caps)
    chunk_lists = [
        _chunks_for(caps[k], k == K - 1, first_phase=(k == 0)) for k in range(K)
    ]

    nc = bacc.Bacc("TRN2", target_bir_lowering=False, debug=False, num_devices=N_CORES)
    # x is stored chunk-major on the host: per chunk a contiguous
    # [P, KO1, tok] block (partition-major: KO1*tok*2-byte DMA rows)
    xts = [
        nc.dram_tensor(f"xt{k}", [P * KO1 * caps[k]], f16, kind="ExternalInput").ap()
        for k in range(K)
    ]
    w1 = nc.dram_tensor("w1", [K, DIM, EXPERT_DIM], f16, kind="ExternalInput").ap()
    b1 = nc.dram_tensor("b1", [K, P, MO1], f32, kind="ExternalInput").ap()
    w2 = nc.dram_tensor("w2", [K, EXPERT_DIM, DIM], f16, kind="ExternalInput").ap()
    gts = [
        nc.dram_tensor(f"g{k}", [P, caps[k]], f32, kind="ExternalInput").ap()
        for k in range(K)
    ]
    yts = [
        nc.dram_tensor(f"yt{k}", [DIM, caps[k]], f16, kind="ExternalOutput").ap()
        for k in range(K)
    ]

    gelu = ACT_FUNC or mybir.ActivationFunctionType.Gelu
    half_cols = (MO1 // 2) * P

    def x_chunk_ap(k, off, tok):
        base = P * KO1 * off
        return xts[k][base : base + P * KO1 * tok].rearrange(
            "(p ko n) -> p ko n", p=P, ko=KO1
        )

    with tile.TileContext(nc) as tc, ExitStack() as ctx:
        w1_pool = ctx.enter_context(tc.tile_pool(name="w1", bufs=2 * KO1))
        w2_pool = ctx.enter_context(tc.tile_pool(name="w2", bufs=KO2 + 6))
        b1_pool = ctx.enter_context(tc.tile_pool(name="b1", bufs=2))
        x_pool = ctx.enter_context(tc.tile_pool(name="x", bufs=3))
        g_pool = ctx.enter_context(tc.tile_pool(name="g", bufs=6))
        h_pool = ctx.enter_context(tc.tile_pool(name="h", bufs=3))
        y_pool = ctx.enter_context(tc.tile_pool(name="y", bufs=10))
        wu_pool = ctx.enter_context(tc.tile_pool(name="wu", bufs=1))
        ps_pool = ctx.enter_context(tc.tile_pool(name="ps", bufs=GRP, space="PSUM"))

        def ps_tile(tok, name):
            # full-bank allocation so matmul outputs never straddle banks
            t = ps_pool.tile([P, 512], f32, tag="ps", name=name)
            return t[:, :tok]

        # PE warmup: ramp the tensor-engine p-state while the first weight/x
        # DMAs are in flight. Garbage values; consumed by a throwaway copy.
        wu_t = wu_pool.tile([P, P], f16)
        nc.vector.memset(wu_t[:], 0.0)
        wu_ps = ps_tile(P, "wu_ps")
        for i in range(WARMUP_MM):
            nc.tensor.matmul(
                wu_ps,
                wu_t[:],
                wu_t[:],
                start=(i == 0),
                stop=(i == WARMUP_MM - 1),
            )
        nc.vector.tensor_copy(wu_t[:], wu_ps)

        # per-phase resources, loaded by _load_phase
        w1_sls = {}
        w2_sls = {}
        x_tss = {}
        g_tss = {}
        b1_ts = {}

        def _load_phase(k):
            """Emit phase k's input DMAs, all on the Sync DGE in consumption
            order. The in-order queue gives the weight stream its priority
            for free; no explicit deps, so no wait can head-block another
            engine's queue. Scalar carries only activations and y writes."""
            chunks = chunk_lists[k]
            n_ch = len(chunks)
            offs = [sum(chunks[:i]) for i in range(n_ch)]
            x_ts = [
                x_pool.tile([P, KO1, chunks[t]], f16, tag="x", name=f"x_{k}_{t}")
                for t in range(n_ch)
            ]
            b1_t = b1_pool.tile([P, MO1], f32)
            w1_sl = []
            if k == 0:
                # critical startup stream: interleave w1 half-A and chunk-0 x
                # ko slices in stage-1 consumption order, then half-B
                x0_ap = x_chunk_ap(0, 0, chunks[0])
                for ko in range(KO1):
                    w = w1_pool.tile([P, EXPERT_DIM], f16, tag="w1")
                    nc.sync.dma_start(
                        w[:, :half_cols], w1[k, ko * P : (ko + 1) * P, :half_cols]
                    )
                    nc.sync.dma_start(x_ts[0][:, ko], x0_ap[:, ko])
                    w1_sl.append(w)
                nc.sync.dma_start(b1_t[:], b1[k])
                for ko in range(KO1):
                    nc.sync.dma_start(
                        w1_sl[ko][:, half_cols:],
                        w1[k, ko * P : (ko + 1) * P, half_cols:],
                    )
                for t in range(1, n_ch):
                    nc.sync.dma_start(x_ts[t][:], x_chunk_ap(k, offs[t], chunks[t]))
            else:
                nc.sync.dma_start(b1_t[:], b1[k])
                for t in range(n_ch):
                    nc.sync.dma_start(x_ts[t][:], x_chunk_ap(k, offs[t], chunks[t]))
                for ko in range(KO1):
                    w = w1_pool.tile([P, EXPERT_DIM], f16, tag="w1")
                    nc.sync.dma_start(w[:], w1[k, ko * P : (ko + 1) * P, :])
                    w1_sl.append(w)
            g_ts = []
            for t in range(n_ch):
                g_t = g_pool.tile([P, chunks[t]], f32, tag="g", name=f"g_{k}_{t}")
                nc.sync.dma_start(g_t[:], gts[k][:, offs[t] : offs[t] + chunks[t]])
                g_ts.append(g_t)
            w2_sl = []
            for ko in range(KO2):
                w = w2_pool.tile([P, DIM], f16, tag="w2")
                nc.sync.dma_start(w[:], w2[k, ko * P : (ko + 1) * P, :])
                w2_sl.append(w)
            w1_sls[k] = w1_sl
            w2_sls[k] = w2_sl
            x_tss[k] = x_ts
            g_tss[k] = g_ts
            b1_ts[k] = b1_t

        _load_phase(0)
        for k in range(K):
            cap = caps[k]
            chunks = chunk_lists[k]
            n_ch = len(chunks)
            offs = [sum(chunks[:i]) for i in range(n_ch)]
            w1_sl = w1_sls[k]
            w2_sl = w2_sls[k]
            x_ts = x_tss[k]
            g_ts = g_tss[k]
            b1_t = b1_ts[k]

            # --- stage 1 over all chunks: h^T = gelu(w1^T @ x^T + b1) ---
            h_ts = []
            G1 = 4  # 4-bank groups: acts of one group drain while the next
            for t in range(n_ch):  # group computes on the other 4 banks
                tok = chunks[t]
                h_t = h_pool.tile([P, MO1, tok], f16, tag="h", name=f"h_{k}_{t}")
                for grp in range(MO1 // G1):
                    pss = [
                        ps_tile(tok, f"ps_{k}_{t}_{grp}_{i}") for i in range(G1)
                    ]
                    for ko in range(KO1):
                        for i in range(G1):
                            mo = grp * G1 + i
                            nc.tensor.matmul(
                                pss[i],
                                w1_sl[ko][:, mo * P : (mo + 1) * P],
                                x_ts[t][:, ko],
                                start=(ko == 0),
                                stop=(ko == KO1 - 1),
                            )
                    for i in range(G1):
                        mo = grp * G1 + i
                        nc.scalar.activation(
                            h_t[:, mo], pss[i], gelu, bias=b1_t[:, mo : mo + 1]
                        )
                h_ts.append(h_t)

            # next phase's inputs: emitted between stage 1 and stage 2 so the
            # weight issues precede stage-2's scalar-side y traffic and the
            # serialized sync weight stream is never blocked
            if k + 1 < K:
                _load_phase(k + 1)

            # --- stage 2 over all chunks: y^T = gate * (w2^T @ h^T) ---
            for t in range(n_ch):
                tok = chunks[t]
                tsl = slice(offs[t], offs[t] + tok)
                h_t = h_ts[t]
                g_t = g_ts[t]
                last = k == K - 1 and t == n_ch - 1
                if last:
                    # m-outer so early m tiles' gate-mul + y DMA overlap the
                    # remaining matmuls (short tail)
                    for mo in range(MO2):
                        ps2 = ps_tile(tok, f"ps2_{k}_{t}_{mo}")
                        for ko in range(KO2):
                            nc.tensor.matmul(
                                ps2,
                                w2_sl[ko][:, mo * P : (mo + 1) * P],
                                h_t[:, ko],
                                start=(ko == 0),
                                stop=(ko == KO2 - 1),
                            )
                        y_t = y_pool.tile(
                            [P, tok], f16, tag="y", name=f"y_{k}_{t}_{mo}"
                        )
                        nc.vector.tensor_mul(y_t[:], ps2, g_t[:])
                        nc.scalar.dma_start(yts[k][mo * P : (mo + 1) * P, tsl], y_t[:])
                else:
                    G2 = MO2 // 2
                    for half2 in range(2):
                        pss2 = [
                            ps_tile(tok, f"ps2_{k}_{t}_{half2}_{i}")
                            for i in range(G2)
                        ]
                        for ko in range(KO2):
                            for i in range(G2):
                                mo = half2 * G2 + i
                                nc.tensor.matmul(
                                    pss2[i],
                                    w2_sl[ko][:, mo * P : (mo + 1) * P],
                                    h_t[:, ko],
                                    start=(ko == 0),
                                    stop=(ko == KO2 - 1),
                                )
                        for i in range(G2):
                            mo = half2 * G2 + i
                            y_t = y_pool.tile(
                                [P, tok], f16, tag="y", name=f"y_{k}_{t}_{mo}"
                            )
                            nc.vector.tensor_mul(y_t[:], pss2[i], g_t[:])
                            nc.scalar.dma_start(
                                yts[k][mo * P : (mo + 1) * P, tsl], y_t[:]
                            )

    nc.compile()
    return nc


def _get_nc(caps):
    key = tuple(caps)
    if key not in _NC_CACHE:
        _NC_CACHE[key] = _build_nc(key)
    return _NC_CACHE[key]


def _solve_pieces(counts, split_set):
    """Cut the experts in split_set into 2 pieces each (others stay whole) and
    group the 8*K pieces into K groups of 8 by size rank. Local search on cut
    positions minimizes sum of group maxima (= per-core compiled work).
    Returns (caps, pieces) with pieces = [[expert, size], ...]."""
    pieces = []
    pair_idx = {}
    for e, c in enumerate(counts):
        if e in split_set:
            a = c // 2
            pair_idx[e] = (len(pieces), len(pieces) + 1)
            pieces.append([e, c - a])
            pieces.append([e, a])
        else:
            pieces.append([e, c])
    K = len(pieces) // 8

    def group_cost(pl):
        s = sorted((sz for _, sz in pl), reverse=True)
        return sum(s[8 * g] for g in range(K))

    for _ in range(300):
        base = group_cost(pieces)
        improved = False
        for e in split_set:
            i1, i2 = pair_idx[e]
            tot = pieces[i1][1] + pieces[i2][1]
            lo = MIN_CHUNK if tot >= 2 * MIN_CHUNK else tot // 2
            for a in range(max(lo, tot // 2 - 250), min(tot - lo, tot // 2 + 251), 4):
                old = (pieces[i1][1], pieces[i2][1])
                pieces[i1][1], pieces[i2][1] = tot - a, a
                c = group_cost(pieces)
                if c < base:
                    base = c
                    improved = True
                else:
                    pieces[i1][1], pieces[i2][1] = old
        if not improved:
            break
    s = sorted((sz for _, sz in pieces), reverse=True)
    caps = [max(s[8 * g], 16) for g in range(K)]
    return caps, pieces


def _solve_slots(counts, n_split=None):
    """Choose which experts to split (8 -> K=3 phases keeps weight traffic at
    24MB/core; 16 -> K=4 minimizes padding) and build the slot assignment.
    Returns (caps, assignment): assignment[core][phase] = (expert, lo, hi)
    token-range claim (hi-lo may be < cap -> zero-padded) or None.
    """
    import os
    import random

    E_ = len(counts)
    if n_split is None:
        n_split = 2 * E_ - 8 * int(os.environ.get("MOE_PHASES", "3"))
        n_split = min(max(n_split, 0), E_)
    if (E_ + n_split) % 8:
        n_split = E_  # fall back to all-split

    order = sorted(range(E_), key=lambda e: -counts[e])
    best = None
    cands = [set(order[:n_split])]
    rng = random.Random(0)
    if 0 < n_split < E_:
        for _ in range(200):
            cands.append(set(rng.sample(range(E_), n_split)))
    for ss in cands:
        caps, pieces = _solve_pieces(counts, ss)
        pad = 8 * sum(caps) - sum(counts)
        if best is None or pad < best[0]:
            best = (pad, caps, pieces)
    _, caps, pieces = best
    K = len(caps)

    porder = sorted(range(len(pieces)), key=lambda i: -pieces[i][1])
    assign = [[None] * K for _ in range(N_CORES)]
    offsets = [0] * E_
    slot_of = {}
    for r, pi in enumerate(porder):
        slot_of[pi] = (r % 8, r // 8)
    for pi, (e, sz) in enumerate(pieces):
        core, ph = slot_of[pi]
        lo = offsets[e]
        offsets[e] = lo + sz
        assign[core][ph] = (e, lo, lo + sz)
    return caps, assign


def _install_ntff_hook():
    """Register the axon NTFF profile hook if the image's antenv lacks it."""
    import sys
    import types

    try:
        from antenv.axon_hooks import get_axon_ntff_profile_hook  # noqa: F401

        return True
    except ImportError:
        pass
    try:
        from trn_agent_boot.trn_boot import _ntff_profile_via_ctypes

        hook = _ntff_profile_via_ctypes("/opt/axon/libaxon_pjrt.so")
        if hook is None:
            return False
        mod = types.ModuleType("antenv.axon_hooks")
        state = {"hook": hook}
        mod.set_axon_ntff_profile_hook = lambda h: state.__setitem__("hook", h)
        mod.get_axon_ntff_profile_hook = lambda: state["hook"]
        sys.modules["antenv.axon_hooks"] = mod
        return True
    except Exception:
        return False


def kernel(x, routing_tensor, w1, b1, w2):
    global LAST_EXEC_NS, LAST_TRACE_PATH
    x = np.ascontiguousarray(np.asarray(x, np.float32))
    routing_tensor = np.asarray(routing_tensor, np.float32)
    w1 = np.asarray(w1, np.float32)
    b1 = np.asarray(b1, np.float32)
    w2 = np.asarray(w2, np.float32)

    idx_list = [np.nonzero(routing_tensor[:, e])[0] for e in range(E)]
    counts = [len(i) for i in idx_list]
    caps, assign = _solve_slots(counts)
    K = len(caps)
    chunk_lists = [
        _chunks_for(caps[k], k == K - 1, first_phase=(k == 0)) for k in range(K)
    ]

    x16 = x.astype(np.float16)
    w1_16 = w1.astype(np.float16)
    w2_16 = w2.astype(np.float16)

    in_maps = []
    for c in range(N_CORES):
        m = {
            "w1": np.zeros((K, DIM, EXPERT_DIM), np.float16),
            "b1": np.zeros((K, P, MO1), np.float32),
            "w2": np.zeros((K, EXPERT_DIM, DIM), np.float16),
        }
        for k in range(K):
            cap = caps[k]
            xt = np.zeros((P, KO1, cap), np.float16)  # sliced per chunk below
            g = np.zeros((P, cap), np.float32)
            slot = assign[c][k]
            if slot is not None:
                e, lo, hi = slot
                idx = idx_list[e][lo:hi]
                # [P, KO1, n]: element (p, ko, t) = x[token_t, ko*P + p]
                xt[:, :, : hi - lo] = (
                    x16[idx].T.reshape(KO1, P, hi - lo).transpose(1, 0, 2)
                )
                g[:, : hi - lo] = routing_tensor[idx, e][None, :]
                m["w1"][k] = w1_16[e]
                m["w2"][k] = w2_16[e]
                m["b1"][k] = b1[e].reshape(MO1, P).T
            # chunk-major flat layout: per chunk a contiguous [P, KO1, tok]
            blocks = []
            off = 0
            for tok in chunk_lists[k]:
                blocks.append(np.ascontiguousarray(xt[:, :, off : off + tok]).ravel())
                off += tok
            m[f"xt{k}"] = np.concatenate(blocks)
            m[f"g{k}"] = g
        in_maps.append(m)

    nc = _get_nc(caps)
    core_ids = list(range(N_CORES))
    if TRACE and _install_ntff_hook():
        import concourse.bass_utils as _bu

        _bu.upload_artifacts = lambda tmpdir: tmpdir  # zero-egress container
        try:
            res = run_bass_kernel_spmd(nc, in_maps, core_ids, trace=True)
            LAST_EXEC_NS = res.exec_time_ns
            LAST_TRACE_PATH = (
                res.instructions_and_trace[1] if res.instructions_and_trace else None
            )
        except Exception:
            res = run_bass_kernel_spmd(nc, in_maps, core_ids)
    else:
        res = run_bass_kernel_spmd(nc, in_maps, core_ids)

    out = np.zeros((N, DIM), np.float32)
    for c in range(N_CORES):
        for k in range(K):
            slot = assign[c][k]
            if slot is None:
                continue
            e, lo, hi = slot
            idx = idx_list[e][lo:hi]
            yt = res.results[c][f"yt{k}"]  # [DIM, cap] f16
            out[idx] += yt[:, : hi - lo].T.astype(np.float32)

    return out


# revision 12
# speedup vs baseline: 1.0271x; 1.0271x over previous
"""MoE expert FFN (forward_all + top-2 routing combine) on 8 TRN2 NeuronCores.

Strategy: the routing tensor has exactly TOP_K=2 nonzeros per token, so only
routed (token, expert) pairs contribute. We dispatch on the host and run the
expert GEMMs expert-parallel on 8 cores. The device program is a sequence of
K "phases"; each phase is a (weights, tokens) GEMM pair of a fixed compiled
capacity, and the host assigns any expert's weights + routed tokens to any
(core, phase) slot. Because the program is compiled per observed routing
counts (cached by capacity signature), experts can be SPLIT across slots and
capacities chosen so padding is ~1% instead of padding every expert to a
global cap. A small solver cuts each expert into two pieces and groups the
8*K pieces by size rank, minimizing the sum of group maxima.

Each phase computes y^T = gate * (w2^T @ gelu(w1^T @ x^T + b1)) with tokens
on the matmul free dim, f16 matmuls (f32 PSUM accumulate, bias+gelu in f32,
gate-mul on DVE), k-interleaved over 8 PSUM banks. Stage 1 runs over ALL of
a phase's chunks before stage 2 starts, so the w1 stream has the whole
stage-1 span to land and stage-2's w2 streams during stage 1. All matmul
operands are 2-D full-tile slices (3-D mid-tensor slices measurably slow the
PE by ~20%). Phase 0's w1 half-columns + per-ko x interleave on the Sync DGE
in consumption order; later phases prefetch in the background (x/gates/b1 on
the Scalar DGE, weights on Sync) dep-chained behind the previous phase's
weight stream. A short warmup matmul block ramps the PE p-state during the
initial DMA wait. PSUM tiles are allocated full-bank so matmul outputs never
straddle banks. y is written f16.
"""

import math
from contextlib import ExitStack

import numpy as np

import concourse.mybir as mybir
import concourse.tile as tile
from concourse import bacc
from concourse.bass_utils import run_bass_kernel_spmd

N, DIM, E, EXPERT_DIM = 8192, 1024, 16, 2048
N_CORES = 8
P = 128

KO1 = DIM // P  # 8 contraction tiles, stage 1
MO1 = EXPERT_DIM // P  # 16 output tiles, stage 1
KO2 = EXPERT_DIM // P  # 16 contraction tiles, stage 2
MO2 = DIM // P  # 8 output tiles, stage 2

GRP = 8  # stage-1 psum group = all 8 PSUM banks
MIN_CHUNK = 240  # below this, ldweights (~97ns) outruns the matmul free dim

TRACE = False  # set by test.py to capture an NTFF profile
LAST_EXEC_NS = None
LAST_TRACE_PATH = None
ACT_FUNC = None  # default Gelu; sim_check overrides (CoreSim lacks Gelu)
WARMUP_MM = 8  # matmuls ramping the PE p-state during the startup DMA wait

_NC_CACHE = {}


def _chunks_for(cap, last_phase, first_phase=False):
    """Split a phase capacity into matmul chunks <=512 (one PSUM bank of f32),
    each >=MIN_CHUNK when possible. Phase 0 gets a maximal first chunk (the
    bigger the chunk, the slower stage 1 consumes each w1 k-slice, covering
    the DMA ramp). The very last chunk of the last phase is kept small-ish so
    the post-matmul tail (gate-mul + y DMA) is short."""
    if cap <= 512:
        if last_phase and cap >= 2 * MIN_CHUNK:
            return [cap - MIN_CHUNK, MIN_CHUNK]
        return [cap]
    if first_phase and cap >= 512 + MIN_CHUNK:
        rest = cap - 512
        n = max(1, -(-rest // 512))
        while n > 1 and rest // n < MIN_CHUNK:
            n -= 1
        ch = [512] + [rest // n + (1 if i < rest % n else 0) for i in range(n)]
        if min(ch) >= MIN_CHUNK:
            return ch
    n = -(-cap // 512)  # ceil
    while True:
        base = cap // n
        if base >= MIN_CHUNK or n == 2:
            break
        n -= 1
    ch = [base + (1 if i < cap % n else 0) for i in range(n)]
    ch.sort(reverse=True)
    if last_phase and len(ch) > 1:
        tail = min(ch[-1], 256)
        rest = cap - tail
        m = len(ch) - 1
        ch = [rest // m + (1 if i < rest % m else 0) for i in range(m)]
        ch.sort(reverse=True)
        ch.append(tail)
    return ch


def _build_nc(caps):
    """caps: tuple of per-phase token capacities (descending-ish)."""
    f32 = mybir.dt.float32
    f16 = mybir.dt.float16
    K = len(caps)
    chunk_lists = [
        _chunks_for(caps[k], k == K - 1, first_phase=(k == 0)) for k in range(K)
    ]

    nc = bacc.Bacc("TRN2", target_bir_lowering=False, debug=False, num_devices=N_CORES)
    # x is stored chunk-major on the host: per chunk a contiguous
    # [P, KO1, tok] block (partition-major: KO1*tok*2-byte DMA rows)
    xts = [
        nc.dram_tensor(f"xt{k}", [P * KO1 * caps[k]], f16, kind="ExternalInput").ap()
        for k in range(K)
    ]
    w1 = nc.dram_tensor("w1", [K, DIM, EXPERT_DIM], f16, kind="ExternalInput").ap()
    b1 = nc.dram_tensor("b1", [K, P, MO1], f32, kind="ExternalInput").ap()
    w2 = nc.dram_tensor("w2", [K, EXPERT_DIM, DIM], f16, kind="ExternalInput").ap()
    gts = [
        nc.dram_tensor(f"g{k}", [P, caps[k]], f32, kind="ExternalInput").ap()
        for k in range(K)
    ]
    yts = [
        nc.dram_tensor(f"yt{k}", [DIM, caps[k]], f16, kind="ExternalOutput").ap()
        for k in range(K)
    ]

    gelu = ACT_FUNC or mybir.ActivationFunctionType.Gelu
    half_cols = (MO1 // 2) * P

    def x_chunk_ap(k, off, tok):
        base = P * KO1 * off
        return xts[k][base : base + P * KO1 * tok].rearrange(
            "(p ko n) -> p ko n", p=P, ko=KO1
        )

    with tile.TileContext(nc) as tc, ExitStack() as ctx:
        w1_pool = ctx.enter_context(tc.tile_pool(name="w1", bufs=2 * KO1))
        w2_pool = ctx.enter_context(tc.tile_pool(name="w2", bufs=KO2 + 6))
        b1_pool = ctx.enter_context(tc.tile_pool(name="b1", bufs=2))
        x_pool = ctx.enter_context(tc.tile_pool(name="x", bufs=3))
        g_pool = ctx.enter_context(tc.tile_pool(name="g", bufs=6))
        h_pool = ctx.enter_context(tc.tile_pool(name="h", bufs=3))
        y_pool = ctx.enter_context(tc.tile_pool(name="y", bufs=10))
        wu_pool = ctx.enter_context(tc.tile_pool(name="wu", bufs=1))
        ps_pool = ctx.enter_context(tc.tile_pool(name="ps", bufs=GRP, space="PSUM"))

        def ps_tile(tok, name):
            # full-bank allocation so matmul outputs never straddle banks
            t = ps_pool.tile([P, 512], f32, tag="ps", name=name)
            return t[:, :tok]

        # PE warmup: ramp the tensor-engine p-state while the first weight/x
        # DMAs are in flight. Garbage values; consumed by a throwaway copy.
        wu_t = wu_pool.tile([P, P], f16)
        nc.vector.memset(wu_t[:], 0.0)
        wu_ps = ps_tile(P, "wu_ps")
        for i in range(WARMUP_MM):
            nc.tensor.matmul(
                wu_ps,
                wu_t[:],
                wu_t[:],
                start=(i == 0),
                stop=(i == WARMUP_MM - 1),
            )
        nc.vector.tensor_copy(wu_t[:], wu_ps)

        # per-phase resources, loaded by _load_phase
        w1_sls = {}
        w2_sls = {}
        x_tss = {}
        g_tss = {}
        b1_ts = {}

        def _load_phase(k):
            """Emit phase k's input DMAs, all on the Sync DGE in consumption
            order. The in-order queue gives the weight stream its priority
            for free; no explicit deps, so no wait can head-block another
            engine's queue. Scalar carries only activations and y writes."""
            chunks = chunk_lists[k]
            n_ch = len(chunks)
            offs = [sum(chunks[:i]) for i in range(n_ch)]
            x_ts = [
                x_pool.tile([P, KO1, chunks[t]], f16, tag="x", name=f"x_{k}_{t}")
                for t in range(n_ch)
            ]
            b1_t = b1_pool.tile([P, MO1], f32)
            w1_sl = []
            if k == 0:
                # critical startup stream: interleave w1 half-A and chunk-0 x
                # ko slices in stage-1 consumption order, then half-B
                x0_ap = x_chunk_ap(0, 0, chunks[0])
                for ko in range(KO1):
                    w = w1_pool.tile([P, EXPERT_DIM], f16, tag="w1")
                    nc.sync.dma_start(
                        w[:, :half_cols], w1[k, ko * P : (ko + 1) * P, :half_cols]
                    )
                    nc.sync.dma_start(x_ts[0][:, ko], x0_ap[:, ko])
                    w1_sl.append(w)
                nc.sync.dma_start(b1_t[:], b1[k])
                for ko in range(KO1):
                    nc.sync.dma_start(
                        w1_sl[ko][:, half_cols:],
                        w1[k, ko * P : (ko + 1) * P, half_cols:],
                    )
                for t in range(1, n_ch):
                    nc.sync.dma_start(x_ts[t][:], x_chunk_ap(k, offs[t], chunks[t]))
            else:
                nc.sync.dma_start(b1_t[:], b1[k])
                for t in range(n_ch):
                    nc.sync.dma_start(x_ts[t][:], x_chunk_ap(k, offs[t], chunks[t]))
                for ko in range(KO1):
                    w = w1_pool.tile([P, EXPERT_DIM], f16, tag="w1")
                    nc.sync.dma_start(w[:], w1[k, ko * P : (ko + 1) * P, :])
                    w1_sl.append(w)
            g_ts = []
            for t in range(n_ch):
                g_t = g_pool.tile([P, chunks[t]], f32, tag="g", name=f"g_{k}_{t}")
                nc.sync.dma_start(g_t[:], gts[k][:, offs[t] : offs[t] + chunks[t]])
                g_ts.append(g_t)
            w2_sl = []
            for ko in range(KO2):
                w = w2_pool.tile([P, DIM], f16, tag="w2")
                nc.sync.dma_start(w[:], w2[k, ko * P : (ko + 1) * P, :])
                w2_sl.append(w)
            w1_sls[k] = w1_sl
            w2_sls[k] = w2_sl
            x_tss[k] = x_ts
            g_tss[k] = g_ts
            b1_ts[k] = b1_t

        _load_phase(0)
        for k in range(K):
            cap = caps[k]
            chunks = chunk_lists[k]
            n_ch = len(chunks)
            offs = [sum(chunks[:i]) for i in range(n_ch)]
            w1_sl = w1_sls[k]
            w2_sl = w2_sls[k]
            x_ts = x_tss[k]
            g_ts = g_tss[k]
            b1_t = b1_ts[k]

            # --- stage 1 over all chunks: h^T = gelu(w1^T @ x^T + b1) ---
            h_ts = []
            G1 = GRP
            for t in range(n_ch):
                tok = chunks[t]
                h_t = h_pool.tile([P, MO1, tok], f16, tag="h", name=f"h_{k}_{t}")
                for grp in range(MO1 // G1):
                    pss = [
                        ps_tile(tok, f"ps_{k}_{t}_{grp}_{i}") for i in range(G1)
                    ]
                    for ko in range(KO1):
                        for i in range(G1):
                            mo = grp * G1 + i
                            nc.tensor.matmul(
                                pss[i],
                                w1_sl[ko][:, mo * P : (mo + 1) * P],
                                x_ts[t][:, ko],
                                start=(ko == 0),
                                stop=(ko == KO1 - 1),
                            )
                    for i in range(G1):
                        mo = grp * G1 + i
                        nc.scalar.activation(
                            h_t[:, mo], pss[i], gelu, bias=b1_t[:, mo : mo + 1]
                        )
                h_ts.append(h_t)

            # next phase's inputs: emitted between stage 1 and stage 2 so the
            # weight issues precede stage-2's scalar-side y traffic and the
            # serialized sync weight stream is never blocked
            if k + 1 < K:
                _load_phase(k + 1)

            # --- stage 2 over all chunks: y^T = gate * (w2^T @ h^T) ---
            for t in range(n_ch):
                tok = chunks[t]
                tsl = slice(offs[t], offs[t] + tok)
                h_t = h_ts[t]
                g_t = g_ts[t]
                last = k == K - 1 and t == n_ch - 1
                if last:
                    # m-outer so early m tiles' gate-mul + y DMA overlap the
                    # remaining matmuls (short tail)
                    for mo in range(MO2):
                        ps2 = ps_tile(tok, f"ps2_{k}_{t}_{mo}")
                        for ko in range(KO2):
                            nc.tensor.matmul(
                                ps2,
                                w2_sl[ko][:, mo * P : (mo + 1) * P],
                                h_t[:, ko],
                                start=(ko == 0),
                                stop=(ko == KO2 - 1),
                            )
                        y_t = y_pool.tile(
                            [P, tok], f16, tag="y", name=f"y_{k}_{t}_{mo}"
                        )
                        nc.vector.tensor_mul(y_t[:], ps2, g_t[:])
                        nc.scalar.dma_start(yts[k][mo * P : (mo + 1) * P, tsl], y_t[:])
                else:
                    G2 = MO2 // 2
                    for half2 in range(2):
                        pss2 = [
                            ps_tile(tok, f"ps2_{k}_{t}_{half2}_{i}")
                            for i in range(G2)
                        ]
                        for ko in range(KO2):
                            for i in range(G2):
                                mo = half2 * G2 + i
                                nc.tensor.matmul(
                                    pss2[i],
                                    w2_sl[ko][:, mo * P : (mo + 1) * P],
                                    h_t[:, ko],
                                    start=(ko == 0),
                                    stop=(ko == KO2 - 1),
                                )
                        for i in range(G2):
                            mo = half2 * G2 + i
                            y_t = y_pool.tile(
                                [P, tok], f16, tag="y", name=f"y_{k}_{t}_{mo}"
                            )
                            nc.vector.tensor_mul(y_t[:], pss2[i], g_t[:])
                            nc.scalar.dma_start(
                                yts[k][mo * P : (mo + 1) * P, tsl], y_t[:]
                            )

    nc.compile()
    return nc


def _get_nc(caps):
    key = tuple(caps)
    if key not in _NC_CACHE:
        _NC_CACHE[key] = _build_nc(key)
    return _NC_CACHE[key]


def _solve_pieces(counts, split_set):
    """Cut the experts in split_set into 2 pieces each (others stay whole) and
    group the 8*K pieces into K groups of 8 by size rank. Local search on cut
    positions minimizes sum of group maxima (= per-core compiled work).
    Returns (caps, pieces) with pieces = [[expert, size], ...]."""
    pieces = []
    pair_idx = {}
    for e, c in enumerate(counts):
        if e in split_set:
            a = c // 2
            pair_idx[e] = (len(pieces), len(pieces) + 1)
            pieces.append([e, c - a])
            pieces.append([e, a])
        else:
            pieces.append([e, c])
    K = len(pieces) // 8

    def group_cost(pl):
        s = sorted((sz for _, sz in pl), reverse=True)
        return sum(s[8 * g] for g in range(K))

    for _ in range(300):
        base = group_cost(pieces)
        improved = False
        for e in split_set:
            i1, i2 = pair_idx[e]
            tot = pieces[i1][1] + pieces[i2][1]
            lo = MIN_CHUNK if tot >= 2 * MIN_CHUNK else tot // 2
            for a in range(max(lo, tot // 2 - 250), min(tot - lo, tot // 2 + 251), 4):
                old = (pieces[i1][1], pieces[i2][1])
                pieces[i1][1], pieces[i2][1] = tot - a, a
                c = group_cost(pieces)
                if c < base:
                    base = c
                    improved = True
                else:
                    pieces[i1][1], pieces[i2][1] = old
        if not improved:
            break
    s = sorted((sz for _, sz in pieces), reverse=True)
    caps = [max(s[8 * g], 16) for g in range(K)]
    return caps, pieces


def _solve_slots(counts, n_split=None):
    """Choose which experts to split (8 -> K=3 phases keeps weight traffic at
    24MB/core; 16 -> K=4 minimizes padding) and build the slot assignment.
    Returns (caps, assignment): assignment[core][phase] = (expert, lo, hi)
    token-range claim (hi-lo may be < cap -> zero-padded) or None.
    """
    import os
    import random

    E_ = len(counts)
    if n_split is None:
        n_split = 2 * E_ - 8 * int(os.environ.get("MOE_PHASES", "3"))
        n_split = min(max(n_split, 0), E_)
    if (E_ + n_split) % 8:
        n_split = E_  # fall back to all-split

    order = sorted(range(E_), key=lambda e: -counts[e])
    best = None
    cands = [set(order[:n_split])]
    rng = random.Random(0)
    if 0 < n_split < E_:
        for _ in range(1000):
            cands.append(set(rng.sample(range(E_), n_split)))
    for ss in cands:
        caps, pieces = _solve_pieces(counts, ss)
        pad = 8 * sum(caps) - sum(counts)
        if best is None or pad < best[0]:
            best = (pad, caps, pieces)
    _, caps, pieces = best
    K = len(caps)

    porder = sorted(range(len(pieces)), key=lambda i: -pieces[i][1])
    assign = [[None] * K for _ in range(N_CORES)]
    offsets = [0] * E_
    slot_of = {}
    for r, pi in enumerate(porder):
        slot_of[pi] = (r % 8, r // 8)
    for pi, (e, sz) in enumerate(pieces):
        core, ph = slot_of[pi]
        lo = offsets[e]
        offsets[e] = lo + sz
        assign[core][ph] = (e, lo, lo + sz)
    return caps, assign


def _install_ntff_hook():
    """Register the axon NTFF profile hook if the image's antenv lacks it."""
    import sys
    import types

    try:
        from antenv.axon_hooks import get_axon_ntff_profile_hook  # noqa: F401

        return True
    except ImportError:
        pass
    try:
        from trn_agent_boot.trn_boot import _ntff_profile_via_ctypes

        hook = _ntff_profile_via_ctypes("/opt/axon/libaxon_pjrt.so")
        if hook is None:
            return False
        mod = types.ModuleType("antenv.axon_hooks")
        state = {"hook": hook}
        mod.set_axon_ntff_profile_hook = lambda h: state.__setitem__("hook", h)
        mod.get_axon_ntff_profile_hook = lambda: state["hook"]
        sys.modules["antenv.axon_hooks"] = mod
        return True
    except Exception:
        return False


def kernel(x, routing_tensor, w1, b1, w2):
    global LAST_EXEC_NS, LAST_TRACE_PATH
    x = np.ascontiguousarray(np.asarray(x, np.float32))
    routing_tensor = np.asarray(routing_tensor, np.float32)
    w1 = np.asarray(w1, np.float32)
    b1 = np.asarray(b1, np.float32)
    w2 = np.asarray(w2, np.float32)

    idx_list = [np.nonzero(routing_tensor[:, e])[0] for e in range(E)]
    counts = [len(i) for i in idx_list]
    caps, assign = _solve_slots(counts)
    K = len(caps)
    chunk_lists = [
        _chunks_for(caps[k], k == K - 1, first_phase=(k == 0)) for k in range(K)
    ]

    x16 = x.astype(np.float16)
    w1_16 = w1.astype(np.float16)
    w2_16 = w2.astype(np.float16)

    in_maps = []
    for c in range(N_CORES):
        m = {
            "w1": np.zeros((K, DIM, EXPERT_DIM), np.float16),
            "b1": np.zeros((K, P, MO1), np.float32),
            "w2": np.zeros((K, EXPERT_DIM, DIM), np.float16),
        }
        for k in range(K):
            cap = caps[k]
            xt = np.zeros((P, KO1, cap), np.float16)  # sliced per chunk below
            g = np.zeros((P, cap), np.float32)
            slot = assign[c][k]
            if slot is not None:
                e, lo, hi = slot
                idx = idx_list[e][lo:hi]
                # [P, KO1, n]: element (p, ko, t) = x[token_t, ko*P + p]
                xt[:, :, : hi - lo] = (
                    x16[idx].T.reshape(KO1, P, hi - lo).transpose(1, 0, 2)
                )
                g[:, : hi - lo] = routing_tensor[idx, e][None, :]
                m["w1"][k] = w1_16[e]
                m["w2"][k] = w2_16[e]
                m["b1"][k] = b1[e].reshape(MO1, P).T
            # chunk-major flat layout: per chunk a contiguous [P, KO1, tok]
            blocks = []
            off = 0
            for tok in chunk_lists[k]:
                blocks.append(np.ascontiguousarray(xt[:, :, off : off + tok]).ravel())
                off += tok
            m[f"xt{k}"] = np.concatenate(blocks)
            m[f"g{k}"] = g
        in_maps.append(m)

    nc = _get_nc(caps)
    core_ids = list(range(N_CORES))
    if TRACE and _install_ntff_hook():
        import concourse.bass_utils as _bu

        _bu.upload_artifacts = lambda tmpdir: tmpdir  # zero-egress container
        try:
            res = run_bass_kernel_spmd(nc, in_maps, core_ids, trace=True)
            LAST_EXEC_NS = res.exec_time_ns
            LAST_TRACE_PATH = (
                res.instructions_and_trace[1] if res.instructions_and_trace else None
            )
        except Exception:
            res = run_bass_kernel_spmd(nc, in_maps, core_ids)
    else:
        res = run_bass_kernel_spmd(nc, in_maps, core_ids)

    out = np.zeros((N, DIM), np.float32)
    for c in range(N_CORES):
        for k in range(K):
            slot = assign[c][k]
            if slot is None:
                continue
            e, lo, hi = slot
            idx = idx_list[e][lo:hi]
            yt = res.results[c][f"yt{k}"]  # [DIM, cap] f16
            out[idx] += yt[:, : hi - lo].T.astype(np.float32)

    return out
